# revision 1
# baseline (speedup 1.0000x reference)
"""Trainium2 kernel for BinaryLinear: out = x @ sign(clip(weight,-1,1)).T + bias.

Full shapes: x [8192, 4096] f32, weight [4096, 4096] f32, bias [4096] f32,
out [8192, 4096] f32.

Strategy (8 NeuronCores, no collectives needed):
  - Grid-shard tokens x out_features across the 8 cores; each core computes
    a disjoint output tile, host slices inputs / stitches outputs.
  - Binarized weights are exactly +-1 (bf16/f32r-exact). The matmul runs
    on the PE at 1 cycle/row using float32r operands (f32 bits, reduced-
    precision multiplier, ~2^-13 per-term error -> ~1e-4 rel overall).
  - Host packs x transposed+tiled so the contraction dim (in_features)
    lands on SBUF partitions with every DMA contiguous at line rate.
  - Per core: resident binarized-transposed weight slice in SBUF,
    stream 128-token blocks of xT, accumulate over K=4096 in PSUM,
    add bias on DVE while copying PSUM->SBUF, DMA out.

MODE:
  "f32r"  : single pass, f32r x f32r, 2x4 grid (tok x outf). ~1e-4 rel.
  "bf16x2": x split hi/lo into two bf16 passes, 4x2 grid. ~2e-6 rel,
            about 1.8x slower.
"""

import sys

if "/opt/trn_rl_repo" not in sys.path:
    sys.path.insert(0, "/opt/trn_rl_repo")

import ml_dtypes
import numpy as np

MODE = "f32r"

N_TOK, D_IN, D_OUT = 8192, 4096, 4096
if MODE == "f32r":
    TOK_SHARDS, OUT_SHARDS = 2, 4
else:
    TOK_SHARDS, OUT_SHARDS = 4, 2
N_CORES = TOK_SHARDS * OUT_SHARDS
TOK_C = N_TOK // TOK_SHARDS
OUT_C = D_OUT // OUT_SHARDS
MB = TOK_C // 128  # token blocks per core
KB = D_IN // 128  # contraction blocks
NF = 512  # matmul moving free dim (one fp32 PSUM bank)
NB = OUT_C // NF  # PSUM banks per token block

_cached_nc = None


def build_nc():
    import concourse.bacc as bacc
    import concourse.mybir as mybir
    import concourse.tile as tile

    dt = mybir.dt
    split = MODE == "bf16x2"
    mdt = dt.bfloat16 if split else dt.float32r

    nc = bacc.Bacc()
    xh_d = nc.dram_tensor("xh", [MB, 128, D_IN], mdt, kind="ExternalInput")
    if split:
        xl_d = nc.dram_tensor("xl", [MB, 128, D_IN], mdt, kind="ExternalInput")
    # weights always ship as bf16 (+-1 is exact); the f32r path upconverts
    # on-chip (DVE) so the weight prefetch moves half the bytes.
    wt_d = nc.dram_tensor("wt", [KB, 128, OUT_C], dt.bfloat16, kind="ExternalInput")
    br_d = nc.dram_tensor("br", [128, OUT_C], dt.float32, kind="ExternalInput")
    out_d = nc.dram_tensor("out", [TOK_C, OUT_C], dt.float32, kind="ExternalOutput")

    # First TRICKLE token-blocks are loaded before the weight stream and
    # their matmuls interleaved per k-block, so the PE computes while
    # weights arrive instead of idling at kernel start.
    TRICKLE = 0 if split else 4

    with tile.TileContext(nc) as tc:
        with (
            tc.tile_pool(name="wts", bufs=1) as wpool,
            tc.tile_pool(name="wstage", bufs=2) as spool,
            tc.tile_pool(name="bias", bufs=1) as bpool,
            tc.tile_pool(name="xin", bufs=max(2, TRICKLE)) as xpool,
            tc.tile_pool(name="outp", bufs=1 if not split else 2) as opool,
            tc.tile_pool(name="psum", bufs=8, space="PSUM") as ppool,
        ):

            def load_x(m):
                xh_m = xpool.tile([128, D_IN], mdt, name=f"xh_{m}", tag="xh")
                nc.sync.dma_start(xh_m[:], xh_d[m])
                passes = [xh_m]
                if split:
                    xl_m = xpool.tile([128, D_IN], mdt, name=f"xl_{m}", tag="xl")
                    nc.sync.dma_start(xl_m[:], xl_d[m])
                    passes.append(xl_m)
                return passes

            def alloc_ps(m):
                return [
                    ppool.tile([128, NF], dt.float32, name=f"ps_{m}_{n}", tag="ps")
                    for n in range(NB)
                ]

            def emit_mms(kb, passes, ps):
                n_half = len(passes)
                for hi, xm in enumerate(passes):
                    lhs = xm[:, kb * 128 : (kb + 1) * 128]
                    for n in range(NB):
                        rhs = wts[kb][:, n * NF : (n + 1) * NF]
                        nc.tensor.matmul(
                            ps[n][:],
                            lhs,
                            rhs,
                            start=(kb == 0 and hi == 0),
                            stop=(kb == KB - 1 and hi == n_half - 1),
                        )

            def flush(m, ps):
                out_t = opool.tile([128, OUT_C], dt.float32, name=f"o_{m}", tag="out")
                for n in range(NB):
                    nc.vector.tensor_tensor(
                        out_t[:, n * NF : (n + 1) * NF],
                        ps[n][:],
                        bias_s[:, n * NF : (n + 1) * NF],
                        mybir.AluOpType.add,
                    )
                nc.sync.dma_start(out_d[m * 128 : (m + 1) * 128, :], out_t[:])

            def load_w(kb):
                if split:
                    w = wpool.tile([128, OUT_C], mdt, name=f"wt{kb}", tag=f"wt{kb}")
                    nc.sync.dma_start(w[:], wt_d[kb])
                else:
                    # bf16 DMA + DVE upconvert; matmul bitcasts to f32r
                    stage = spool.tile(
                        [128, OUT_C], dt.bfloat16, name=f"ws{kb}", tag="wstage"
                    )
                    nc.sync.dma_start(stage[:], wt_d[kb])
                    w = wpool.tile(
                        [128, OUT_C], dt.float32r, name=f"wt{kb}", tag=f"wt{kb}"
                    )
                    nc.vector.tensor_copy(w[:], stage[:])
                wts.append(w)

            # Interleave trickle-x loads with the weight stream so both the
            # PE's first operands and the early k-blocks arrive ASAP.
            wts = []
            trickle_x = {}
            if TRICKLE:
                trickle_x[0] = load_x(0)
                for kb in range(0, 6):
                    load_w(kb)
                trickle_x[1] = load_x(1)
                for kb in range(6, 14):
                    load_w(kb)
                trickle_x[2] = load_x(2)
                for kb in range(14, KB):
                    load_w(kb)
            else:
                for kb in range(KB):
                    load_w(kb)
            bias_s = bpool.tile([128, OUT_C], dt.float32, name="bias_s")
            nc.sync.dma_start(bias_s[:], br_d[:])
            for m in range(3, TRICKLE):
                trickle_x[m] = load_x(m)

            if TRICKLE:
                trickle_ps = {m: alloc_ps(m) for m in range(TRICKLE)}
                # m-major kb-chunks ordered to match DMA arrivals of
                # (xt_m, wt[kb]) so the PE never waits on a late tile.
                sched = [
                    (0, 0, 6),
                    (1, 0, 6),
                    (0, 6, 14),
                    (1, 6, 14),
                    (2, 0, 14),
                    (0, 14, KB),
                    (1, 14, KB),
                    (2, 14, KB),
                ] + [(m, 0, KB) for m in range(3, TRICKLE)]
                for m, k0, k1 in sched:
                    for kb in range(k0, k1):
                        emit_mms(kb, trickle_x[m], trickle_ps[m])
                for m in range(TRICKLE):
                    flush(m, trickle_ps[m])

            for m in range(TRICKLE, MB):
                passes = load_x(m)
                ps = alloc_ps(m)
                for kb in range(KB):
                    emit_mms(kb, passes, ps)
                flush(m, ps)

    nc.compile()
    return nc


def _pack_x(a):
    """[TOK_C, D_IN] -> [MB, 128, D_IN] with layout [m, p, (kb t)]:
    packed[m, p, kb*128 + t] = a[m*128 + t, kb*128 + p]."""
    return np.ascontiguousarray(
        a.reshape(MB, 128, KB, 128).transpose(0, 3, 2, 1)
    ).reshape(MB, 128, D_IN)


def prepare_in_maps(x, weight, bias):
    x = np.asarray(x, dtype=np.float32)
    weight = np.asarray(weight, dtype=np.float32)
    bias = np.asarray(bias, dtype=np.float32)
    split = MODE == "bf16x2"
    npdt = ml_dtypes.bfloat16 if split else np.float32

    bw = np.where(weight >= 0, np.float32(1.0), np.float32(-1.0))

    wt_packs, bias_packs = [], []
    for oi in range(OUT_SHARDS):
        w_sh = bw[oi * OUT_C : (oi + 1) * OUT_C]  # [OUT_C, D_IN]
        wt = np.ascontiguousarray(w_sh.T).astype(ml_dtypes.bfloat16)
        wt_packs.append(wt.reshape(KB, 128, OUT_C))
        bias_packs.append(
            np.ascontiguousarray(
                np.broadcast_to(bias[oi * OUT_C : (oi + 1) * OUT_C], (128, OUT_C))
            )
        )

    xh_packs, xl_packs = [], []
    for ti in range(TOK_SHARDS):
        x_sh = x[ti * TOK_C : (ti + 1) * TOK_C]
        if split:
            xh = x_sh.astype(ml_dtypes.bfloat16)
            xh_packs.append(_pack_x(xh))
            xl = (x_sh - xh.astype(np.float32)).astype(ml_dtypes.bfloat16)
            xl_packs.append(_pack_x(xl))
        else:
            xh_packs.append(_pack_x(x_sh))

    in_maps = []
    for c in range(N_CORES):
        ti, oi = divmod(c, OUT_SHARDS)
        m = {"xh": xh_packs[ti], "wt": wt_packs[oi], "br": bias_packs[oi]}
        if split:
            m["xl"] = xl_packs[ti]
        in_maps.append(m)
    return in_maps


def run(in_maps, trace=False, **kwargs):
    global _cached_nc
    from concourse.bass_utils import run_bass_kernel_spmd

    if _cached_nc is None:
        _cached_nc = build_nc()
    return run_bass_kernel_spmd(
        _cached_nc, in_maps, list(range(N_CORES)), trace=trace, **kwargs
    )


def gather(results):
    out = np.empty((N_TOK, D_OUT), dtype=np.float32)
    for c in range(N_CORES):
        ti, oi = divmod(c, OUT_SHARDS)
        out[ti * TOK_C : (ti + 1) * TOK_C, oi * OUT_C : (oi + 1) * OUT_C] = results[c][
            "out"
        ]
    return out


def kernel(x, weight, bias):
    res = run(prepare_in_maps(x, weight, bias), trace=False)
    return gather(res.results)



# revision 2
# speedup vs baseline: 1.4460x; 1.4460x over previous
"""Trainium2 kernel for BinaryLinear: out = x @ sign(clip(weight,-1,1)).T + bias.

Full shapes: x [8192, 4096] f32, weight [4096, 4096] f32, bias [4096] f32,
out [8192, 4096] f32.

Strategy (8 NeuronCores, no collectives):
  - Grid-shard tokens x out_features (4x2); each core computes a disjoint
    output tile, host slices inputs / stitches outputs.
  - Weights binarize to exactly +-1, which fp8 e4m3 represents exactly, so
    the matmul can run in fp8 with perf_mode=DoubleRow: 2 fp8 weights per
    PE cell, 256-deep contraction per instruction, ~1.5-2x the bf16/f32r
    row rate.
  - x is quantized to e4m3 (rel err ~2.6e-2 per element). That alone gives
    ~2.8e-2 max rel output error, above the 2e-2 budget, so the first
    CORR/16ths of the contraction also accumulate an e4m3-quantized
    residual pass (x - e4m3(x)), reusing the same sign tiles. Output error
    scales as 2.8e-2 * sqrt(1 - CORR/16).
  - Per core: resident fp8 sign pairs in SBUF, stream 128-token blocks of
    packed fp8 x pairs (stationary), accumulate K in PSUM, bias-add on DVE,
    DMA out.
"""

import sys

if "/opt/trn_rl_repo" not in sys.path:
    sys.path.insert(0, "/opt/trn_rl_repo")

import ml_dtypes
import numpy as np

N_TOK, D_IN, D_OUT = 8192, 4096, 4096
TOK_SHARDS, OUT_SHARDS = 4, 2
N_CORES = TOK_SHARDS * OUT_SHARDS
TOK_C = N_TOK // TOK_SHARDS
OUT_C = D_OUT // OUT_SHARDS
MB = TOK_C // 128  # token blocks per core
KBP = D_IN // 256  # contraction pair-blocks (256 logical k each)
NF = 512  # matmul moving free dim (one fp32 PSUM bank)
NB = OUT_C // NF  # PSUM banks per token block
CORR = 12  # pair-blocks (of KBP) that also get a residual pass

FP8 = ml_dtypes.float8_e4m3  # TRN float8e4 semantics (inf at 256, max 240)

_cached_nc = None


def build_nc():
    import concourse.bacc as bacc
    import concourse.mybir as mybir
    import concourse.tile as tile

    dt = mybir.dt
    DR = mybir.MatmulPerfMode.DoubleRow

    nc = bacc.Bacc()
    xq_d = nc.dram_tensor("xq", [MB, 128, 2 * KBP, 128], dt.float8e4, kind="ExternalInput")
    if CORR:
        xr_d = nc.dram_tensor(
            "xr", [MB, 128, 2 * CORR, 128], dt.float8e4, kind="ExternalInput"
        )
    wt_d = nc.dram_tensor("wt", [KBP, 128, 2, OUT_C], dt.float8e4, kind="ExternalInput")
    br_d = nc.dram_tensor("br", [128, OUT_C], dt.float32, kind="ExternalInput")
    out_d = nc.dram_tensor("out", [TOK_C, OUT_C], dt.float32, kind="ExternalOutput")

    with tile.TileContext(nc) as tc:
        with (
            tc.tile_pool(name="wts", bufs=1) as wpool,
            tc.tile_pool(name="bias", bufs=1) as bpool,
            tc.tile_pool(name="xin", bufs=3) as xpool,
            tc.tile_pool(name="xres", bufs=3) as rpool,
            tc.tile_pool(name="outp", bufs=2) as opool,
            tc.tile_pool(name="psum", bufs=8, space="PSUM") as ppool,
        ):

            def load_x(m):
                xq_m = xpool.tile([128, 2 * KBP, 128], dt.float8e4, name=f"xq_{m}", tag="xq")
                nc.sync.dma_start(xq_m[:], xq_d[m])
                if CORR:
                    xr_m = rpool.tile(
                        [128, 2 * CORR, 128], dt.float8e4, name=f"xr_{m}", tag="xr"
                    )
                    nc.sync.dma_start(xr_m[:], xr_d[m])
                else:
                    xr_m = None
                return xq_m, xr_m

            def alloc_ps(m):
                return [
                    ppool.tile([128, NF], dt.float32, name=f"ps_{m}_{n}", tag="ps")
                    for n in range(NB)
                ]

            def emit_mms(m, xq_m, xr_m, ps):
                for kb in range(KBP):
                    lhs = xq_m[:, 2 * kb : 2 * kb + 2, :]
                    for n in range(NB):
                        rhs = wts[kb][:, :, n * NF : (n + 1) * NF]
                        nc.tensor.matmul(
                            ps[n][:],
                            lhs,
                            rhs,
                            start=(kb == 0),
                            stop=(kb == KBP - 1 and CORR == 0),
                            perf_mode=DR,
                        )
                for kb in range(CORR):
                    lhs = xr_m[:, 2 * kb : 2 * kb + 2, :]
                    for n in range(NB):
                        rhs = wts[kb][:, :, n * NF : (n + 1) * NF]
                        nc.tensor.matmul(
                            ps[n][:],
                            lhs,
                            rhs,
                            start=False,
                            stop=(kb == CORR - 1),
                            perf_mode=DR,
                        )

            def flush(m, ps):
                out_t = opool.tile([128, OUT_C], dt.float32, name=f"o_{m}", tag="out")
                for n in range(NB):
                    nc.vector.tensor_tensor(
                        out_t[:, n * NF : (n + 1) * NF],
                        ps[n][:],
                        bias_s[:, n * NF : (n + 1) * NF],
                        mybir.AluOpType.add,
                    )
                nc.sync.dma_start(out_d[m * 128 : (m + 1) * 128, :], out_t[:])

            # x for the first token block first, so the PE can start as soon
            # as the first sign tiles land; then the full sign stream.
            first_x = load_x(0)
            wts = []
            for kb in range(KBP):
                w = wpool.tile([128, 2, OUT_C], dt.float8e4, name=f"wt{kb}", tag=f"wt{kb}")
                nc.sync.dma_start(w[:], wt_d[kb])
                wts.append(w)
            bias_s = bpool.tile([128, OUT_C], dt.float32, name="bias_s")
            nc.sync.dma_start(bias_s[:], br_d[:])

            for m in range(MB):
                xq_m, xr_m = first_x if m == 0 else load_x(m)
                ps = alloc_ps(m)
                emit_mms(m, xq_m, xr_m, ps)
                flush(m, ps)

    nc.compile()
    return nc


def _pack_pairs(a, kbp):
    """[TOK_C, kbp*256] fp8 -> [MB, 128, 2*kbp, 128] with
    packed[m, p, 2*kb + i, t] = a[m*128 + t, kb*256 + i*128 + p]."""
    mb = a.shape[0] // 128
    return np.ascontiguousarray(
        a.reshape(mb, 128, kbp, 2, 128).transpose(0, 4, 2, 3, 1)
    ).reshape(mb, 128, 2 * kbp, 128)


def prepare_in_maps(x, weight, bias):
    x = np.asarray(x, dtype=np.float32)
    weight = np.asarray(weight, dtype=np.float32)
    bias = np.asarray(bias, dtype=np.float32)

    bw = np.where(weight >= 0, np.float32(1.0), np.float32(-1.0))

    wt_packs, bias_packs = [], []
    for oi in range(OUT_SHARDS):
        s_sh = bw[oi * OUT_C : (oi + 1) * OUT_C]  # [OUT_C, D_IN]
        # wt[kb, p, i, o] = s[o, kb*256 + i*128 + p]
        wt = np.ascontiguousarray(
            s_sh.T.reshape(KBP, 2, 128, OUT_C).transpose(0, 2, 1, 3)
        ).astype(FP8)
        wt_packs.append(wt)
        bias_packs.append(
            np.ascontiguousarray(
                np.broadcast_to(bias[oi * OUT_C : (oi + 1) * OUT_C], (128, OUT_C))
            )
        )

    xq_packs, xr_packs = [], []
    for ti in range(TOK_SHARDS):
        x_sh = x[ti * TOK_C : (ti + 1) * TOK_C]
        xq = x_sh.astype(FP8)
        xq_packs.append(_pack_pairs(xq, KBP))
        if CORR:
            res = x_sh[:, : CORR * 256] - xq[:, : CORR * 256].astype(np.float32)
            xr_packs.append(_pack_pairs(res.astype(FP8), CORR))

    in_maps = []
    for c in range(N_CORES):
        ti, oi = divmod(c, OUT_SHARDS)
        m = {"xq": xq_packs[ti], "wt": wt_packs[oi], "br": bias_packs[oi]}
        if CORR:
            m["xr"] = xr_packs[ti]
        in_maps.append(m)
    return in_maps


def run(in_maps, trace=False, **kwargs):
    global _cached_nc
    from concourse.bass_utils import run_bass_kernel_spmd

    if _cached_nc is None:
        _cached_nc = build_nc()
    return run_bass_kernel_spmd(
        _cached_nc, in_maps, list(range(N_CORES)), trace=trace, **kwargs
    )


def gather(results):
    out = np.empty((N_TOK, D_OUT), dtype=np.float32)
    for c in range(N_CORES):
        ti, oi = divmod(c, OUT_SHARDS)
        out[ti * TOK_C : (ti + 1) * TOK_C, oi * OUT_C : (oi + 1) * OUT_C] = results[c][
            "out"
        ]
    return out


def kernel(x, weight, bias):
    res = run(prepare_in_maps(x, weight, bias), trace=False)
    return gather(res.results)
